# revision 8
# baseline (speedup 1.0000x reference)
"""Trainium2 Bass kernel for CompatV1LSTM.

Reference computation (per batch row b):
    h_0 = c_0 = 0
    for t in 0..T-1:
        z = [x_t, h] @ kernel + bias            # [B, 4H], gates (i, j, f, o)
        c = c * sigmoid(f + 1.0) + sigmoid(i) * tanh(j)
        h = tanh(c) * sigmoid(o)
    y = h @ w_out + b_out                       # [B, C]

Sharding: data-parallel over batch across 8 NeuronCores (64 rows/core).
LSTM weights / output head replicated.

Per-core design:
  - z computed in [B_local=64 (partitions), 4H=1024 (free)] layout via PE
    matmuls: lhsT = [x_t; h]^T chunks [128, 64] (stationary), rhs = kernel
    chunks [128, 512] (moving), fp32r (full PE rate at N=512, ~tf32 accuracy).
  - Gate columns permuted to [f | i | j | o] at weight-load time so each gate
    nonlinearity is one contiguous ACT instruction, ordered (f, i, j, o) to
    unblock the DVE cell-state update earliest. FORGET_BIAS applied via the
    ACT bias operand on the f sigmoid.
  - When the bias / b_out inputs are all-zero (as in setup_inputs), the bias
    matmuls are dropped entirely; otherwise they are applied via a padded
    K=128 matmul with a one-hot lhsT column.
  - x_t^T tiles produced on-device by PE transposes of DMA'd x groups,
    prefetched ahead of the recurrence; both 128-halves transpose into one
    [128, 128] PSUM tile, evacuated with a single DVE copy.
  - h^T for the next step's matmuls produced the same way (2 PE transposes
    into one PSUM tile + 1 DVE copy).
  - sigmoid(i)*tanh(j) runs on GPSIMD to take it off the DVE critical path.
"""

import numpy as np

B, T, D, H, C = 512, 128, 256, 256, 128
NCORES = 8
BL = B // NCORES  # 64 batch rows per core
FORGET_BIAS = 1.0
XG = 4   # timesteps per x DMA group
PF = 2   # groups prefetched ahead

_CACHE: dict = {}


def _build_program(with_bias: bool, with_out_bias: bool):
    from contextlib import ExitStack

    import concourse.mybir as mybir
    import concourse.tile as tile
    from concourse import bacc
    from concourse.masks import make_identity

    f32 = mybir.dt.float32
    f32r = mybir.dt.float32r
    AF = mybir.ActivationFunctionType

    nc = bacc.Bacc(
        "TRN2",
        target_bir_lowering=False,
        debug=False,
        enable_asserts=False,
        num_devices=NCORES,
    )

    x_d = nc.dram_tensor("x", (BL, T, D), f32, kind="ExternalInput").ap()
    k_d = nc.dram_tensor("kernel", (D + H, 4 * H), f32, kind="ExternalInput").ap()
    b_d = nc.dram_tensor("bias", (4 * H,), f32, kind="ExternalInput").ap()
    wo_d = nc.dram_tensor("w_out", (H, C), f32, kind="ExternalInput").ap()
    bo_d = nc.dram_tensor("b_out", (C,), f32, kind="ExternalInput").ap()
    y_d = nc.dram_tensor("y", (BL, C), f32, kind="ExternalOutput").ap()

    # gate column permutation: dst block -> src block, dst order (f, i, j, o),
    # src order (i, j, f, o)
    PERM = [(0, 2), (1, 0), (2, 1), (3, 3)]
    SF, SI, TJ, SO = (slice(b * H, (b + 1) * H) for b in range(4))

    with tile.TileContext(nc) as tc, ExitStack() as ctx:
        persist = ctx.enter_context(tc.tile_pool(name="persist", bufs=1))
        xg_pool = ctx.enter_context(tc.tile_pool(name="xg", bufs=PF + 1))
        xq_pool = ctx.enter_context(tc.tile_pool(name="xq", bufs=(PF + 1) * XG))
        gates = ctx.enter_context(tc.tile_pool(name="gates", bufs=2))
        hpool = ctx.enter_context(tc.tile_pool(name="hp", bufs=3))
        zpsum = ctx.enter_context(tc.tile_pool(name="zps", bufs=2, space="PSUM"))
        tpsum = ctx.enter_context(tc.tile_pool(name="tps", bufs=3, space="PSUM"))
        opsum = ctx.enter_context(tc.tile_pool(name="ops", bufs=1, space="PSUM"))

        ident = persist.tile([128, 128], f32, name="ident")
        make_identity(nc, ident)

        # memset can't target f32r tiles (invalid ISA value type); stage
        # constants in f32 scratch and DVE-copy (casts) into f32r tiles.
        scratch = persist.tile([128, 4 * H], f32, name="scratch")

        if with_bias or with_out_bias:
            # one-hot column (row 0) used as lhsT for bias-broadcast matmuls
            ones_pad = persist.tile([128, BL], f32r, name="ones_pad")
            nc.vector.memset(scratch[:, :BL], 0.0)
            nc.vector.memset(scratch[0:1, :BL], 1.0)
            nc.vector.tensor_copy(ones_pad, scratch[:, :BL])

        # LSTM kernel, gate-permuted: Wsb[:, kc, :] is rows kc*128..+128
        Wsb = persist.tile([128, 4, 4 * H], f32r, name="Wsb")
        for kc in range(4):
            for dstb, srcb in PERM:
                nc.sync.dma_start(
                    Wsb[:, kc, dstb * H:(dstb + 1) * H],
                    k_d[kc * 128:(kc + 1) * 128,
                        srcb * H:(srcb + 1) * H].bitcast(f32r),
                )

        if with_bias:
            # bias row (padded to K=128), gate-permuted
            bias_pad = persist.tile([128, 4 * H], f32r, name="bias_pad")
            nc.vector.memset(scratch, 0.0)
            for dstb, srcb in PERM:
                nc.sync.dma_start(scratch[0:1, dstb * H:(dstb + 1) * H],
                                  b_d[None, srcb * H:(srcb + 1) * H])
            nc.vector.tensor_copy(bias_pad, scratch)

        # output head
        wout_sb = persist.tile([128, 2, C], f32r, name="wout_sb")
        for kc in range(2):
            nc.sync.dma_start(wout_sb[:, kc],
                              wo_d[kc * 128:(kc + 1) * 128, :].bitcast(f32r))
        if with_out_bias:
            bout_pad = persist.tile([128, C], f32r, name="bout_pad")
            nc.vector.memset(scratch[:, :C], 0.0)
            nc.sync.dma_start(scratch[0:1, :C], bo_d[None, :])
            nc.vector.tensor_copy(bout_pad, scratch[:, :C])

        # recurrent state
        c_t = persist.tile([BL, H], f32, name="c_t")
        nc.vector.memset(c_t, 0.0)
        hT = hpool.tile([128, 2 * BL], f32r, name="ht")
        nc.vector.memset(scratch[:, :2 * BL], 0.0)
        nc.vector.tensor_copy(hT, scratch[:, :2 * BL])

        NG = T // XG
        xT: dict = {}

        def load_group(g):
            xg = xg_pool.tile([BL, XG, D], f32, name="xg")
            nc.sync.dma_start(xg, x_d[:, g * XG:(g + 1) * XG, :])
            for i in range(XG):
                t = g * XG + i
                pt = tpsum.tile([128, 128], f32, name="pt")
                for hh in range(2):
                    nc.tensor.transpose(pt[:, hh * BL:(hh + 1) * BL],
                                        xg[:, i, hh * 128:(hh + 1) * 128],
                                        ident[:BL, :BL])
                xt = xq_pool.tile([128, 2 * BL], f32r, name="xt")
                nc.vector.tensor_copy(xt, pt)
                xT[t] = xt

        def emit_zx(t):
            """bias + x-projection matmuls for step t (independent of h)."""
            zp = zpsum.tile([BL, 4 * H], f32, name="zp")
            xt = xT.pop(t)
            for nh in range(2):
                ns = slice(nh * 512, (nh + 1) * 512)
                if with_bias:
                    nc.tensor.matmul(zp[:, ns], ones_pad, bias_pad[:, ns],
                                     start=True, stop=False)
                nc.tensor.matmul(zp[:, ns], xt[:, :BL], Wsb[:, 0, ns],
                                 start=not with_bias, stop=False)
                nc.tensor.matmul(zp[:, ns], xt[:, BL:], Wsb[:, 1, ns],
                                 start=False, stop=False)
            return zp

        for g in range(PF):
            load_group(g)
        zp_cur = emit_zx(0)

        for t in range(T):
            if t % XG == 0 and t // XG + PF < NG:
                load_group(t // XG + PF)

            # h-projection matmuls complete z for step t
            zp = zp_cur
            for nh in range(2):
                ns = slice(nh * 512, (nh + 1) * 512)
                nc.tensor.matmul(zp[:, ns], hT[:, :BL], Wsb[:, 2, ns],
                                 start=False, stop=False)
                nc.tensor.matmul(zp[:, ns], hT[:, BL:], Wsb[:, 3, ns],
                                 start=False, stop=True)

            # gate nonlinearities (ACT), f first to unblock the c update
            sf = gates.tile([BL, H], f32, name="sf")
            nc.scalar.activation(sf, zp[:, SF], AF.Sigmoid, bias=FORGET_BIAS)
            si = gates.tile([BL, H], f32, name="si")
            nc.scalar.activation(si, zp[:, SI], AF.Sigmoid)
            tj = gates.tile([BL, H], f32, name="tj")
            nc.scalar.activation(tj, zp[:, TJ], AF.Tanh)
            so = gates.tile([BL, H], f32, name="so")
            nc.scalar.activation(so, zp[:, SO], AF.Sigmoid)

            # c = c * sf + si * tj   (mul+add on DVE, si*tj on GPSIMD)
            nc.vector.tensor_mul(out=c_t, in0=c_t, in1=sf)
            t1 = gates.tile([BL, H], f32, name="t1")
            nc.gpsimd.tensor_tensor(t1, si, tj, mybir.AluOpType.mult)
            nc.vector.tensor_add(out=c_t, in0=c_t, in1=t1)

            # h = tanh(c) * so
            th = gates.tile([BL, H], f32, name="th")
            nc.scalar.activation(th, c_t, AF.Tanh)
            h_t = gates.tile([BL, H], f32, name="h_t")
            nc.vector.tensor_mul(out=h_t, in0=th, in1=so)

            # start next step's independent matmuls while h transposes
            if t + 1 < T:
                zp_cur = emit_zx(t + 1)

            # h^T for the next step's lhsT: 2 PE transposes into one PSUM
            # tile + a single DVE copy
            pt = tpsum.tile([128, 128], f32, name="pt")
            for hh in range(2):
                nc.tensor.transpose(pt[:, hh * BL:(hh + 1) * BL],
                                    h_t[:, hh * 128:(hh + 1) * 128],
                                    ident[:BL, :BL])
            hT = hpool.tile([128, 2 * BL], f32r, name="ht")
            nc.vector.tensor_copy(hT, pt)

        # output head: y = h_last @ w_out + b_out
        op = opsum.tile([BL, C], f32, name="op")
        if with_out_bias:
            nc.tensor.matmul(op, ones_pad, bout_pad, start=True, stop=False)
        nc.tensor.matmul(op, hT[:, :BL], wout_sb[:, 0],
                         start=not with_out_bias, stop=False)
        nc.tensor.matmul(op, hT[:, BL:], wout_sb[:, 1], start=False, stop=True)
        y_sb = persist.tile([BL, C], f32, name="y_sb")
        nc.vector.tensor_copy(y_sb, op)
        nc.sync.dma_start(y_d, y_sb)

    nc.compile()
    return nc


def _get_program(with_bias: bool = False, with_out_bias: bool = False):
    key = (with_bias, with_out_bias)
    if key not in _CACHE:
        _CACHE[key] = _build_program(with_bias, with_out_bias)
    return _CACHE[key]


def _run(inputs: dict, trace: bool = False):
    from concourse.bass_utils import run_bass_kernel_spmd

    x = np.ascontiguousarray(np.asarray(inputs["x"], dtype=np.float32))
    shared = {
        "kernel": np.ascontiguousarray(np.asarray(inputs["kernel"], np.float32)),
        "bias": np.ascontiguousarray(np.asarray(inputs["bias"], np.float32)),
        "w_out": np.ascontiguousarray(np.asarray(inputs["w_out"], np.float32)),
        "b_out": np.ascontiguousarray(np.asarray(inputs["b_out"], np.float32)),
    }
    nc = _get_program(bool(np.any(shared["bias"])), bool(np.any(shared["b_out"])))
    in_maps = [
        {"x": np.ascontiguousarray(x[i * BL:(i + 1) * BL]), **shared}
        for i in range(NCORES)
    ]
    res = run_bass_kernel_spmd(nc, in_maps, core_ids=list(range(NCORES)),
                               trace=trace)
    y = np.concatenate([res.results[i]["y"] for i in range(NCORES)], axis=0)
    return y.astype(np.float32), res


def kernel(**inputs) -> np.ndarray:
    y, _ = _run(inputs, trace=False)
    return y
